# revision 17
# baseline (speedup 1.0000x reference)
"""Trainium2 Bass kernel for a 2-layer GMMConv GNN (DGL-style), 8-core SPMD.

Strategy (dst-partitioned, one AllGather per layer):
  - Core c owns nodes [c*6250, (c+1)*6250) and every edge whose dst is owned.
  - Per layer: each core computes h = x @ W for its node slab on the PE
    (bf16), then one AllGather builds the full node-feature table
    [50000, 128]-padded bf16 in every core's HBM.
  - Edge pass: h rows are fetched by src via gpsimd dma_gather (256B rows,
    <=1024 idxs/call, rotating SWDGE queues). The int16 index limit is
    handled with two passes over src halves (<25000 / >=25000) gathering
    from a sliced table base.
  - Scatter: edges are host-sorted by dst block (128 nodes). For each
    128-edge chunk a gw-scaled one-hot [128 edges x 128 dst slots] is built
    on DVE/ACT and a PE matmul accumulates agg into PSUM per dst block;
    blocks flush into an SBUF accumulator (copy on lo pass, add on hi).
  - gw = exp(-0.5 sum_d ((ew_d - mu_d) * isig_d)^2) is computed on device;
    mu/inv_sigma are baked as immediates. Host-side padding edges carry
    ew = 1e4 so gw underflows to exactly 0 and they contribute nothing.
  - Layer 2 repeats the structure (40 outputs), then bias + log_softmax per
    block and each core writes its own [6250, 40] output slab.

Host-side work is limited to index/layout preprocessing (bucketing edges by
owner, sorting by dst block and src half, wrapping index arrays into the
SBUF layouts the hardware ops expect).
"""

import os
import sys

sys.path.insert(0, "/opt/trn_rl_repo")

import numpy as np
import ml_dtypes

from concourse import bass, bacc, tile
from concourse.bass_utils import run_bass_kernel_spmd
import concourse.mybir as mybir

# ---- problem constants (hardcoded per contract) ----
N_NODES = 50000
N_EDGES = 800000
IN_FEATS = 128
N_HIDDEN = 64
OUT_FEATS = 40
DIM = 8
N_CORES = 8
NODES_PER_CORE = N_NODES // N_CORES  # 6250
BLOCKS_PER_CORE = (NODES_PER_CORE + 127) // 128  # 49
NODES_PAD = BLOCKS_PER_CORE * 128  # 6272
HALF = 25000  # src-half split (int16 idx limit)
TBL_COLS = 128  # bf16 -> 256B rows (dma_gather elem granularity)
GBATCH = 512  # idxs per dma_gather call (SWDGE ring is 1024 descs/queue)
NQUEUES = 1

BF16 = mybir.dt.bfloat16
F32 = mybir.dt.float32
I16 = mybir.dt.int16

F32NP = np.float32
BF16NP = ml_dtypes.bfloat16

# fraction of one-hot builds routed to ACT (Square+Exp) instead of DVE
ACT_ONEHOT_EVERY = int(os.environ.get("K_ACT_ONEHOT_EVERY", "3"))  # every Nth chunk on ACT; 0=all DVE


def _preprocess(src, dst, edge_weight):
    """Bucket edges by dst owner; per core sort by (src_half, dst_block);
    build a shared chunk plan and per-core padded, wrapped arrays."""
    src = np.asarray(src).astype(np.int64)
    dst = np.asarray(dst).astype(np.int64)
    ew = np.asarray(edge_weight).astype(np.float32)

    owner = dst // NODES_PER_CORE
    per_core = []
    for c in range(N_CORES):
        m = owner == c
        s, d, w = src[m], dst[m] - c * NODES_PER_CORE, ew[m]
        half = (s >= HALF).astype(np.int64)
        block = d // 128
        key = half * BLOCKS_PER_CORE + block
        order = np.argsort(key, kind="stable")
        per_core.append((s[order], d[order], w[order], key[order]))

    n_groups = 2 * BLOCKS_PER_CORE
    counts = np.zeros((N_CORES, n_groups), np.int64)
    for c in range(N_CORES):
        k = per_core[c][3]
        cnt = np.bincount(k, minlength=n_groups)
        counts[c] = cnt
    # chunks per group: max over cores, at least 1
    gchunks = np.maximum(1, (counts.max(axis=0) + 127) // 128)  # [n_groups]
    C_lo = int(gchunks[:BLOCKS_PER_CORE].sum())
    C_hi = int(gchunks[BLOCKS_PER_CORE:].sum())
    C = C_lo + C_hi
    E_pad = C * 128

    src_adj = np.zeros((N_CORES, E_pad), np.int16)
    # pad edges point one-hot column 128 (never matches iota 0..127) → zero
    # contribution even if gw padding were nonzero
    dst_loc = np.full((N_CORES, E_pad), 128, np.int16)
    ew_pad = np.full((N_CORES, E_pad, DIM), 1.0e4, np.float32)

    g_starts = np.zeros(n_groups + 1, np.int64)
    g_starts[1:] = np.cumsum(gchunks * 128)
    for c in range(N_CORES):
        s, d, w, k = per_core[c]
        e_starts = np.zeros(n_groups + 1, np.int64)
        e_starts[1:] = np.cumsum(counts[c])
        for g in range(n_groups):
            n = counts[c][g]
            o = g_starts[g]
            if n == 0:
                continue
            es = e_starts[g]
            half = g // BLOCKS_PER_CORE
            block = g % BLOCKS_PER_CORE
            src_adj[c, o : o + n] = (s[es : es + n] - half * HALF).astype(np.int16)
            dst_loc[c, o : o + n] = (d[es : es + n] - block * 128).astype(np.int16)
            ew_pad[c, o : o + n] = w[es : es + n]
    # pad edges: src_adj 0, dst_loc 0, ew 1e4 (gw==0)

    # wrapped layouts
    ew_wrapped = ew_pad.reshape(N_CORES, C, 128, DIM).transpose(0, 2, 1, 3).copy()  # [N, 128, C, 8]
    dst_col = dst_loc.reshape(N_CORES, C, 128).transpose(0, 2, 1).astype(np.float32).copy()  # [N, 128, C]
    negdst = -dst_col
    # gather idx arrays: global 16-wrap; calls slice columns
    srcidx = np.tile(
        src_adj.reshape(N_CORES, E_pad // 16, 16).transpose(0, 2, 1), (1, 8, 1)
    ).copy()  # [N, 128, E_pad//16]

    plan = {
        "C": C,
        "C_lo": C_lo,
        "C_hi": C_hi,
        "gchunks": gchunks,  # [2*BLOCKS]
    }
    return plan, src_adj, srcidx, dst_col, negdst, ew_wrapped


_PROGRAM_CACHE = {}


def _build_program_cached(plan, mu1, isig1, mu2, isig2, timing_mode=False):
    key = (
        plan["C"],
        plan["C_lo"],
        plan["gchunks"].tobytes(),
        np.asarray(mu1, np.float64).tobytes(),
        np.asarray(isig1, np.float64).tobytes(),
        np.asarray(mu2, np.float64).tobytes(),
        np.asarray(isig2, np.float64).tobytes(),
        timing_mode,
    )
    if key not in _PROGRAM_CACHE:
        _PROGRAM_CACHE[key] = _build_program(plan, mu1, isig1, mu2, isig2, timing_mode)
    return _PROGRAM_CACHE[key]


def _build_program(plan, mu1, isig1, mu2, isig2, timing_mode=False):
    C = plan["C"]
    C_lo = plan["C_lo"]
    gchunks = plan["gchunks"]

    nc = bacc.Bacc(
        "TRN2",
        target_bir_lowering=False,
        debug=False,
        num_devices=N_CORES,
        num_swdge_queues=NQUEUES,
    )

    feat_ext = nc.declare_dram_parameter("feat", [NODES_PAD, IN_FEATS], F32, isOutput=False)
    ew_ext = nc.declare_dram_parameter("ew", [128, C, DIM], F32, isOutput=False)
    dst_ext = nc.declare_dram_parameter("dstc", [128, C], F32, isOutput=False)
    negdst_ext = nc.declare_dram_parameter("negdst", [128, C], F32, isOutput=False)
    sidx_ext = nc.declare_dram_parameter("sidx", [128, C * 8], I16, isOutput=False)
    w1_ext = nc.declare_dram_parameter("w1", [IN_FEATS, N_HIDDEN], F32, isOutput=False)
    w2_ext = nc.declare_dram_parameter("w2", [N_HIDDEN, OUT_FEATS], F32, isOutput=False)
    b1_ext = nc.declare_dram_parameter("b1b", [128, N_HIDDEN], F32, isOutput=False)
    b2_ext = nc.declare_dram_parameter("b2b", [128, OUT_FEATS], F32, isOutput=False)
    iota_ext = nc.declare_dram_parameter("iota", [128, 128], BF16, isOutput=False)
    ident_ext = nc.declare_dram_parameter("ident", [128, 128], F32, isOutput=False)
    gwb_ext = nc.declare_dram_parameter("gwb", [128, 2 * DIM], F32, isOutput=False)
    out_ext = nc.declare_dram_parameter("out", [NODES_PER_CORE, OUT_FEATS], F32, isOutput=True)

    slab1 = nc.dram_tensor("slab1", [NODES_PER_CORE, TBL_COLS], BF16)
    table1 = nc.dram_tensor("table1", [N_NODES, TBL_COLS], BF16, addr_space="Shared")
    slab2 = nc.dram_tensor("slab2", [NODES_PER_CORE, TBL_COLS], BF16)
    table2 = nc.dram_tensor("table2", [N_NODES, TBL_COLS], BF16, addr_space="Shared")

    # precompute gw scale constants
    s1 = (np.asarray(isig1, np.float64).reshape(DIM) / np.sqrt(2.0)).astype(np.float64)
    m1 = np.asarray(mu1, np.float64).reshape(DIM)
    s2 = (np.asarray(isig2, np.float64).reshape(DIM) / np.sqrt(2.0)).astype(np.float64)
    m2 = np.asarray(mu2, np.float64).reshape(DIM)

    with tile.TileContext(nc) as tc:
        with (
            tc.tile_pool(name="const", bufs=1) as cpool,
            tc.tile_pool(name="edges", bufs=1) as epool,
            tc.tile_pool(name="acc", bufs=1) as apool,
            tc.tile_pool(name="work", bufs=3) as wpool,
            tc.tile_pool(name="gath", bufs=6) as gpool,
            tc.tile_pool(name="oh", bufs=8) as ohpool,
            tc.tile_pool(name="ps", bufs=2, space="PSUM") as pspool,
            tc.tile_pool(name="pst", bufs=2, space="PSUM") as pstpool,
        ):
            # ---- constants / edge data in SBUF ----
            iota = cpool.tile([128, 128], BF16)
            ident = cpool.tile([128, 128], F32)
            w1t = cpool.tile([IN_FEATS, N_HIDDEN], F32)
            w1b = cpool.tile([IN_FEATS, N_HIDDEN], BF16)
            w2t = cpool.tile([N_HIDDEN, OUT_FEATS], F32)
            w2b = cpool.tile([N_HIDDEN, OUT_FEATS], BF16)
            b1b = cpool.tile([128, N_HIDDEN], F32)
            b2b = cpool.tile([128, OUT_FEATS], F32)
            nc.sync.dma_start(iota[:], iota_ext[:])
            nc.sync.dma_start(ident[:], ident_ext[:])
            nc.sync.dma_start(w1t[:], w1_ext[:])
            nc.sync.dma_start(w2t[:], w2_ext[:])
            nc.sync.dma_start(b1b[:], b1_ext[:])
            nc.sync.dma_start(b2b[:], b2_ext[:])
            gwb = cpool.tile([128, 2 * DIM], F32)
            nc.sync.dma_start(gwb[:], gwb_ext[:])
            nc.vector.tensor_copy(w1b[:], w1t[:])
            nc.vector.tensor_copy(w2b[:], w2t[:])

            ewt = epool.tile([128, C, DIM], F32)
            dstc = epool.tile([128, C], F32)
            negdst = epool.tile([128, C], F32)
            sidx = epool.tile([128, C * 8], I16)
            nc.sync.dma_start(ewt[:], ew_ext[:])
            nc.sync.dma_start(dstc[:], dst_ext[:])
            nc.sync.dma_start(negdst[:], negdst_ext[:])
            nc.sync.dma_start(sidx[:], sidx_ext[:])

            acc1 = apool.tile([128, BLOCKS_PER_CORE, N_HIDDEN], F32)
            acc2 = apool.tile([128, BLOCKS_PER_CORE, OUT_FEATS], F32)

            # ---- gw for both layers ----
            def make_gw(mu, sc, tag):
                sqs = epool.tile([128, C, DIM], F32, tag="sqs")
                for d in range(DIM):
                    nc.scalar.activation(
                        sqs[:, :, d],
                        ewt[:, :, d],
                        mybir.ActivationFunctionType.Square,
                        bias=gwb[:, (tag - 1) * DIM + d : (tag - 1) * DIM + d + 1],
                        scale=float(sc[d]),
                    )
                lwn = epool.tile([128, C], F32, tag="lwn")  # -log gw
                nc.vector.tensor_reduce(lwn[:], sqs[:], mybir.AxisListType.X, mybir.AluOpType.add)
                lw = epool.tile([128, C], F32, tag=f"lw{tag}")  # log gw
                nc.vector.tensor_scalar(lw[:], lwn[:], -1.0, None, mybir.AluOpType.mult)
                gw = epool.tile([128, C], F32, tag=f"gw{tag}")
                nc.scalar.activation(gw[:], lwn[:], mybir.ActivationFunctionType.Exp, scale=-1.0)
                return gw, lw

            gw1, lw1 = make_gw(m1, s1, 1)
            gw2, lw2 = make_gw(m2, s2, 2)

            # ---- dense phase: h = x @ W slab, write bf16 table slab ----
            def dense_layer(x_block_fn, w_bf, w_cols, slab_dram, kdim):
                """x_block_fn(b) -> SBUF f32 tile [128, kdim]; writes slab."""
                for b in range(BLOCKS_PER_CORE):
                    xb = x_block_fn(b)
                    xt_ps = pstpool.tile([kdim, 128], F32, tag="tps")
                    nc.tensor.transpose(xt_ps[:], xb, ident[:])
                    xt = wpool.tile([kdim, 128], BF16, tag="xt")
                    nc.scalar.copy(xt[:], xt_ps[:])
                    h_ps = pstpool.tile([128, w_cols], F32, tag="hps")
                    nc.tensor.matmul(h_ps[:], xt[:], w_bf[:], start=True, stop=True)
                    st = wpool.tile([128, TBL_COLS], BF16, tag="slabt")
                    nc.vector.memset(st[:], 0.0)
                    nc.scalar.copy(st[:, 0:w_cols], h_ps[:])
                    rows = min(128, NODES_PER_CORE - b * 128)
                    nc.sync.dma_start(
                        slab_dram[b * 128 : b * 128 + rows, :], st[0:rows, :]
                    )

            # layer-1 dense: x = features
            def feat_block(b):
                ft = wpool.tile([128, IN_FEATS], F32, tag="feat")
                nc.sync.dma_start(ft[:], feat_ext[b * 128 : (b + 1) * 128, :])
                return ft[:]

            dense_layer(feat_block, w1b, N_HIDDEN, slab1, 128)

            def allgather(slab_dram, table_dram):
                if timing_mode:
                    # timing-only stand-in (TimelineSim is single-core):
                    # local copy approximating the AG wall time contribution
                    nc.sync.dma_start(
                        table_dram[0:NODES_PER_CORE, :], slab_dram[:]
                    )
                    return
                nc.gpsimd.collective_compute(
                    "AllGather",
                    mybir.AluOpType.bypass,
                    ins=[slab_dram[:]],
                    outs=[table_dram[:]],
                    replica_groups=[list(range(N_CORES))],
                )

            allgather(slab1, table1)

            # ---- edge phase ----
            def edge_layer(table_dram, gw, lw, acc, w_cols, lname):
                # per pass: lo chunks [0, C_lo), hi chunks [C_lo, C)
                chunk0 = 0
                for p in range(2):
                    pass_chunks = C_lo if p == 0 else plan["C"] - C_lo
                    tbl = table_dram[p * HALF : p * HALF + HALF, :]
                    # gather batches
                    gtiles = []
                    bch = GBATCH // 128
                    nb = (pass_chunks + bch - 1) // bch
                    for k in range(nb):
                        cs = chunk0 + k * bch
                        nch = min(bch, chunk0 + pass_chunks - cs)
                        g = gpool.tile([128, bch, TBL_COLS], BF16, tag="g")
                        nidx = nch * 128
                        nc.gpsimd.dma_gather(
                            g[:, 0:nch, :],
                            tbl,
                            sidx[:, cs * 8 : cs * 8 + nidx // 16],
                            nidx,
                            nidx,
                            TBL_COLS,
                            queue_num=0,
                        )
                        gtiles.append(g)
                    # blocks
                    ci = chunk0
                    for b in range(BLOCKS_PER_CORE):
                        nch = int(gchunks[p * BLOCKS_PER_CORE + b])
                        ps = pspool.tile([128, w_cols], F32, tag="scat")
                        for j in range(nch):
                            c = ci + j
                            oh = ohpool.tile([128, 128], BF16, tag="oh")
                            if ACT_ONEHOT_EVERY and (c % ACT_ONEHOT_EVERY == 0):
                                sq = ohpool.tile([128, 128], BF16, tag="sq")
                                nc.scalar.activation(
                                    sq[:],
                                    iota[:],
                                    mybir.ActivationFunctionType.Square,
                                    bias=negdst[:, c : c + 1],
                                    scale=1.0,
                                )
                                nc.scalar.activation(
                                    oh[:],
                                    sq[:],
                                    mybir.ActivationFunctionType.Exp,
                                    bias=lw[:, c : c + 1],
                                    scale=-30.0,
                                )
                            else:
                                nc.vector.tensor_scalar(
                                    oh[:],
                                    iota[:],
                                    dstc[:, c : c + 1],
                                    gw[:, c : c + 1],
                                    mybir.AluOpType.is_equal,
                                    mybir.AluOpType.mult,
                                )
                            krel = c - chunk0
                            g = gtiles[krel // (GBATCH // 128)]
                            nc.tensor.matmul(
                                ps[:],
                                oh[:],
                                g[:, krel % (GBATCH // 128), 0:w_cols],
                                start=(j == 0),
                                stop=(j == nch - 1),
                            )
                        if p == 0:
                            nc.vector.tensor_copy(acc[:, b, :], ps[:])
                        else:
                            nc.vector.tensor_tensor(
                                acc[:, b, :], acc[:, b, :], ps[:], mybir.AluOpType.add
                            )
                        ci += nch
                    chunk0 += pass_chunks

            edge_layer(table1, gw1, lw1, acc1, N_HIDDEN, "l1")

            # ---- layer-2 dense ----
            def x2_block(b):
                xb = wpool.tile([128, N_HIDDEN], F32, tag="x2")
                nc.vector.tensor_tensor(xb[:], acc1[:, b, :], b1b[:], mybir.AluOpType.add)
                return xb[:]

            dense_layer(x2_block, w2b, OUT_FEATS, slab2, N_HIDDEN)

            allgather(slab2, table2)

            edge_layer(table2, gw2, lw2, acc2, OUT_FEATS, "l2")

            # ---- bias + log_softmax + output ----
            for b in range(BLOCKS_PER_CORE):
                o = wpool.tile([128, OUT_FEATS], F32, tag="o")
                nc.vector.tensor_tensor(o[:], acc2[:, b, :], b2b[:], mybir.AluOpType.add)
                mx = wpool.tile([128, 1], F32, tag="mx")
                nc.vector.tensor_reduce(mx[:], o[:], mybir.AxisListType.X, mybir.AluOpType.max)
                nmx = wpool.tile([128, 1], F32, tag="nmx")
                nc.vector.tensor_scalar(nmx[:], mx[:], -1.0, None, mybir.AluOpType.mult)
                ex = wpool.tile([128, OUT_FEATS], F32, tag="ex")
                se = wpool.tile([128, 1], F32, tag="se")
                nc.scalar.activation(
                    ex[:], o[:], mybir.ActivationFunctionType.Exp,
                    bias=nmx[:], scale=1.0, accum_out=se[:],
                )
                lse = wpool.tile([128, 1], F32, tag="lse")
                nc.scalar.activation(lse[:], se[:], mybir.ActivationFunctionType.Ln)
                res = wpool.tile([128, OUT_FEATS], F32, tag="res")
                nc.vector.tensor_scalar(
                    res[:], o[:], mx[:], lse[:],
                    mybir.AluOpType.subtract, mybir.AluOpType.subtract,
                )
                rows = min(128, NODES_PER_CORE - b * 128)
                nc.sync.dma_start(out_ext[b * 128 : b * 128 + rows, :], res[0:rows, :])

    nc.compile()
    return nc


def _make_in_maps(inp, plan, srcidx, dst_col, negdst, ew_wrapped):
    features = np.asarray(inp["features"], np.float32)
    W1 = np.asarray(inp["W1"], np.float32).reshape(IN_FEATS, N_HIDDEN)
    W2 = np.asarray(inp["W2"], np.float32).reshape(N_HIDDEN, OUT_FEATS)
    b1 = np.asarray(inp["b1"], np.float32).reshape(N_HIDDEN)
    b2 = np.asarray(inp["b2"], np.float32).reshape(OUT_FEATS)

    iota_np = np.tile(np.arange(128, dtype=np.float32), (128, 1)).astype(BF16NP)
    ident_np = np.eye(128, dtype=np.float32)
    b1b = np.tile(b1, (128, 1)).astype(np.float32)
    b2b = np.tile(b2, (128, 1)).astype(np.float32)
    m1 = np.asarray(inp["mu1"], np.float64).reshape(DIM)
    s1 = np.asarray(inp["inv_sigma1"], np.float64).reshape(DIM) / np.sqrt(2.0)
    m2 = np.asarray(inp["mu2"], np.float64).reshape(DIM)
    s2 = np.asarray(inp["inv_sigma2"], np.float64).reshape(DIM) / np.sqrt(2.0)
    gwb_row = np.concatenate([-m1 * s1, -m2 * s2]).astype(np.float32)
    gwb_np = np.tile(gwb_row, (128, 1)).astype(np.float32)

    in_maps = []
    for c in range(N_CORES):
        fs = np.zeros((NODES_PAD, IN_FEATS), np.float32)
        fs[:NODES_PER_CORE] = features[c * NODES_PER_CORE : (c + 1) * NODES_PER_CORE]
        in_maps.append(
            {
                "feat": fs,
                "ew": ew_wrapped[c],
                "dstc": dst_col[c],
                "negdst": negdst[c],
                "sidx": srcidx[c],
                "w1": W1,
                "w2": W2,
                "b1b": b1b,
                "b2b": b2b,
                "iota": iota_np,
                "ident": ident_np,
                "gwb": gwb_np,
            }
        )
    return in_maps


def kernel(
    features,
    edge_weight,
    src,
    dst,
    W1,
    b1,
    mu1,
    inv_sigma1,
    W2,
    b2,
    mu2,
    inv_sigma2,
):
    inp = {
        "features": features,
        "W1": W1,
        "W2": W2,
        "b1": b1,
        "b2": b2,
        "mu1": mu1,
        "inv_sigma1": inv_sigma1,
        "mu2": mu2,
        "inv_sigma2": inv_sigma2,
    }
    plan, src_adj, srcidx, dst_col, negdst, ew_wrapped = _preprocess(src, dst, edge_weight)
    nc = _build_program_cached(plan, mu1, inv_sigma1, mu2, inv_sigma2)
    in_maps = _make_in_maps(inp, plan, srcidx, dst_col, negdst, ew_wrapped)
    res = run_bass_kernel_spmd(nc, in_maps, list(range(N_CORES)))
    out = np.concatenate([res.results[c]["out"] for c in range(N_CORES)], axis=0)
    return out.astype(np.float32)


# revision 19
# speedup vs baseline: 1.3692x; 1.3692x over previous
"""Trainium2 Bass kernel for a 2-layer GMMConv GNN (DGL-style), 8-core SPMD.

Strategy (dst-partitioned, one AllGather per layer):
  - Core c owns nodes [c*6250, (c+1)*6250) and every edge whose dst is owned.
  - Per layer: each core computes h = x @ W for its node slab on the PE
    (bf16), then one AllGather builds the full node-feature table
    [50000, 128]-padded bf16 in every core's HBM.
  - Edge pass: h rows are fetched by src via gpsimd dma_gather (256B rows,
    <=1024 idxs/call, rotating SWDGE queues). The int16 index limit is
    handled with two passes over src halves (<25000 / >=25000) gathering
    from a sliced table base.
  - Scatter: edges are host-sorted by dst block (128 nodes). For each
    128-edge chunk a gw-scaled one-hot [128 edges x 128 dst slots] is built
    on DVE/ACT and a PE matmul accumulates agg into PSUM per dst block;
    blocks flush into an SBUF accumulator (copy on lo pass, add on hi).
  - gw = exp(-0.5 sum_d ((ew_d - mu_d) * isig_d)^2) is computed on device;
    mu/inv_sigma are baked as immediates. Host-side padding edges carry
    ew = 1e4 so gw underflows to exactly 0 and they contribute nothing.
  - Layer 2 repeats the structure (40 outputs), then bias + log_softmax per
    block and each core writes its own [6250, 40] output slab.

Host-side work is limited to index/layout preprocessing (bucketing edges by
owner, sorting by dst block and src half, wrapping index arrays into the
SBUF layouts the hardware ops expect).
"""

import os
import sys

sys.path.insert(0, "/root/.axon_site/_ro/trn_rl_repo")

import numpy as np
import ml_dtypes

from concourse import bass, bacc, tile
from concourse.bass_utils import run_bass_kernel_spmd
import concourse.mybir as mybir

# ---- problem constants (hardcoded per contract) ----
N_NODES = 50000
N_EDGES = 800000
IN_FEATS = 128
N_HIDDEN = 64
OUT_FEATS = 40
DIM = 8
N_CORES = 8
NODES_PER_CORE = N_NODES // N_CORES  # 6250
BLOCKS_PER_CORE = (NODES_PER_CORE + 127) // 128  # 49
NODES_PAD = BLOCKS_PER_CORE * 128  # 6272
HALF = 25000  # src-half split (int16 idx limit)
TBL_COLS = 128  # bf16 -> 256B rows (dma_gather elem granularity)
GBATCH = 2048  # idxs per dma_gather call (single_packet=False required above 1024)
NQUEUES = 1

BF16 = mybir.dt.bfloat16
F32 = mybir.dt.float32
I16 = mybir.dt.int16

F32NP = np.float32
BF16NP = ml_dtypes.bfloat16



def _preprocess(src, dst, edge_weight):
    """Bucket edges by dst owner; per core sort by (src_half, dst_block);
    build a shared chunk plan and per-core padded, wrapped arrays."""
    src = np.asarray(src).astype(np.int64)
    dst = np.asarray(dst).astype(np.int64)
    ew = np.asarray(edge_weight).astype(np.float32)

    owner = dst // NODES_PER_CORE
    per_core = []
    for c in range(N_CORES):
        m = owner == c
        s, d, w = src[m], dst[m] - c * NODES_PER_CORE, ew[m]
        half = (s >= HALF).astype(np.int64)
        block = d // 128
        key = half * BLOCKS_PER_CORE + block
        order = np.argsort(key, kind="stable")
        per_core.append((s[order], d[order], w[order], key[order]))

    n_groups = 2 * BLOCKS_PER_CORE
    counts = np.zeros((N_CORES, n_groups), np.int64)
    for c in range(N_CORES):
        k = per_core[c][3]
        cnt = np.bincount(k, minlength=n_groups)
        counts[c] = cnt
    # chunks per group: max over cores, at least 1
    gchunks = np.maximum(1, (counts.max(axis=0) + 127) // 128)  # [n_groups]
    C_lo = int(gchunks[:BLOCKS_PER_CORE].sum())
    C_hi = int(gchunks[BLOCKS_PER_CORE:].sum())
    C = C_lo + C_hi
    E_pad = C * 128

    src_adj = np.zeros((N_CORES, E_pad), np.int16)
    # pad edges point one-hot column 128 (never matches iota 0..127) → zero
    # contribution even if gw padding were nonzero
    dst_loc = np.full((N_CORES, E_pad), 128, np.int16)
    ew_pad = np.full((N_CORES, E_pad, DIM), 1.0e4, np.float32)

    g_starts = np.zeros(n_groups + 1, np.int64)
    g_starts[1:] = np.cumsum(gchunks * 128)
    for c in range(N_CORES):
        s, d, w, k = per_core[c]
        e_starts = np.zeros(n_groups + 1, np.int64)
        e_starts[1:] = np.cumsum(counts[c])
        for g in range(n_groups):
            n = counts[c][g]
            o = g_starts[g]
            if n == 0:
                continue
            es = e_starts[g]
            half = g // BLOCKS_PER_CORE
            block = g % BLOCKS_PER_CORE
            src_adj[c, o : o + n] = (s[es : es + n] - half * HALF).astype(np.int16)
            dst_loc[c, o : o + n] = (d[es : es + n] - block * 128).astype(np.int16)
            ew_pad[c, o : o + n] = w[es : es + n]
    # pad edges: src_adj 0, dst_loc 0, ew 1e4 (gw==0)

    # wrapped layouts
    ew_wrapped = ew_pad.reshape(N_CORES, C, 128, DIM).transpose(0, 2, 1, 3).copy()  # [N, 128, C, 8]
    dst_col = dst_loc.reshape(N_CORES, C, 128).transpose(0, 2, 1).astype(np.float32).copy()  # [N, 128, C]
    negdst = -dst_col
    # gather idx arrays: global 16-wrap; calls slice columns
    srcidx = np.tile(
        src_adj.reshape(N_CORES, E_pad // 16, 16).transpose(0, 2, 1), (1, 8, 1)
    ).copy()  # [N, 128, E_pad//16]

    plan = {
        "C": C,
        "C_lo": C_lo,
        "C_hi": C_hi,
        "gchunks": gchunks,  # [2*BLOCKS]
    }
    return plan, src_adj, srcidx, dst_col, negdst, ew_wrapped


_PROGRAM_CACHE = {}


def _build_program_cached(plan, mu1, isig1, mu2, isig2, timing_mode=False):
    key = (
        plan["C"],
        plan["C_lo"],
        plan["gchunks"].tobytes(),
        np.asarray(mu1, np.float64).tobytes(),
        np.asarray(isig1, np.float64).tobytes(),
        np.asarray(mu2, np.float64).tobytes(),
        np.asarray(isig2, np.float64).tobytes(),
        timing_mode,
    )
    if key not in _PROGRAM_CACHE:
        _PROGRAM_CACHE[key] = _build_program(plan, mu1, isig1, mu2, isig2, timing_mode)
    return _PROGRAM_CACHE[key]


def _build_program(plan, mu1, isig1, mu2, isig2, timing_mode=False):
    C = plan["C"]
    C_lo = plan["C_lo"]
    gchunks = plan["gchunks"]

    nc = bacc.Bacc(
        "TRN2",
        target_bir_lowering=False,
        debug=False,
        num_devices=N_CORES,
        num_swdge_queues=NQUEUES,
    )

    feat_ext = nc.declare_dram_parameter("feat", [NODES_PAD, IN_FEATS], F32, isOutput=False)
    ew_ext = nc.declare_dram_parameter("ew", [128, C, DIM], F32, isOutput=False)
    dst_ext = nc.declare_dram_parameter("dstc", [128, C], F32, isOutput=False)
    sidx_ext = nc.declare_dram_parameter("sidx", [128, C * 8], I16, isOutput=False)
    w1_ext = nc.declare_dram_parameter("w1", [IN_FEATS, N_HIDDEN], F32, isOutput=False)
    w2_ext = nc.declare_dram_parameter("w2", [N_HIDDEN, OUT_FEATS], F32, isOutput=False)
    b1_ext = nc.declare_dram_parameter("b1b", [128, N_HIDDEN], F32, isOutput=False)
    b2_ext = nc.declare_dram_parameter("b2b", [128, OUT_FEATS], F32, isOutput=False)
    iota_ext = nc.declare_dram_parameter("iota", [128, 128], BF16, isOutput=False)
    ident_ext = nc.declare_dram_parameter("ident", [128, 128], F32, isOutput=False)
    gwb_ext = nc.declare_dram_parameter("gwb", [128, 2 * DIM], F32, isOutput=False)
    out_ext = nc.declare_dram_parameter("out", [NODES_PER_CORE, OUT_FEATS], F32, isOutput=True)

    slab1 = nc.dram_tensor("slab1", [NODES_PER_CORE, TBL_COLS], BF16)
    table1 = nc.dram_tensor("table1", [N_NODES, TBL_COLS], BF16, addr_space="Shared")
    slab2 = nc.dram_tensor("slab2", [NODES_PER_CORE, TBL_COLS], BF16)
    table2 = nc.dram_tensor("table2", [N_NODES, TBL_COLS], BF16, addr_space="Shared")

    # precompute gw scale constants
    s1 = (np.asarray(isig1, np.float64).reshape(DIM) / np.sqrt(2.0)).astype(np.float64)
    m1 = np.asarray(mu1, np.float64).reshape(DIM)
    s2 = (np.asarray(isig2, np.float64).reshape(DIM) / np.sqrt(2.0)).astype(np.float64)
    m2 = np.asarray(mu2, np.float64).reshape(DIM)

    with tile.TileContext(nc) as tc:
        with (
            tc.tile_pool(name="const", bufs=1) as cpool,
            tc.tile_pool(name="edges", bufs=1) as epool,
            tc.tile_pool(name="acc", bufs=1) as apool,
            tc.tile_pool(name="work", bufs=3) as wpool,
            tc.tile_pool(name="gath", bufs=6) as gpool,
            tc.tile_pool(name="oh", bufs=8) as ohpool,
            tc.tile_pool(name="ps", bufs=2, space="PSUM") as pspool,
            tc.tile_pool(name="pst", bufs=2, space="PSUM") as pstpool,
        ):
            # ---- constants / edge data in SBUF ----
            iota = cpool.tile([128, 128], BF16)
            ident = cpool.tile([128, 128], F32)
            w1t = cpool.tile([IN_FEATS, N_HIDDEN], F32)
            w1b = cpool.tile([IN_FEATS, N_HIDDEN], BF16)
            w2t = cpool.tile([N_HIDDEN, OUT_FEATS], F32)
            w2b = cpool.tile([N_HIDDEN, OUT_FEATS], BF16)
            b1b = cpool.tile([128, N_HIDDEN], F32)
            b2b = cpool.tile([128, OUT_FEATS], F32)
            nc.sync.dma_start(iota[:], iota_ext[:])
            nc.sync.dma_start(ident[:], ident_ext[:])
            nc.sync.dma_start(w1t[:], w1_ext[:])
            nc.sync.dma_start(w2t[:], w2_ext[:])
            nc.sync.dma_start(b1b[:], b1_ext[:])
            nc.sync.dma_start(b2b[:], b2_ext[:])
            gwb = cpool.tile([128, 2 * DIM], F32)
            nc.sync.dma_start(gwb[:], gwb_ext[:])
            nc.vector.tensor_copy(w1b[:], w1t[:])
            nc.vector.tensor_copy(w2b[:], w2t[:])

            ewt = epool.tile([128, C, DIM], F32)
            dstc = epool.tile([128, C], F32)
            sidx = epool.tile([128, C * 8], I16)
            nc.sync.dma_start(ewt[:], ew_ext[:])
            nc.sync.dma_start(dstc[:], dst_ext[:])
            nc.sync.dma_start(sidx[:], sidx_ext[:])

            acc1 = apool.tile([128, BLOCKS_PER_CORE, N_HIDDEN], F32)
            acc2 = apool.tile([128, BLOCKS_PER_CORE, OUT_FEATS], F32)

            # ---- gw for both layers ----
            def make_gw(mu, sc, tag):
                sqs = epool.tile([128, C, DIM], F32, tag="sqs")
                for d in range(DIM):
                    nc.scalar.activation(
                        sqs[:, :, d],
                        ewt[:, :, d],
                        mybir.ActivationFunctionType.Square,
                        bias=gwb[:, (tag - 1) * DIM + d : (tag - 1) * DIM + d + 1],
                        scale=float(sc[d]),
                    )
                lwn = epool.tile([128, C], F32, tag="lwn")  # -log gw
                nc.vector.tensor_reduce(lwn[:], sqs[:], mybir.AxisListType.X, mybir.AluOpType.add)
                gw = epool.tile([128, C], F32, tag=f"gw{tag}")
                nc.scalar.activation(gw[:], lwn[:], mybir.ActivationFunctionType.Exp, scale=-1.0)
                return gw

            gw1 = make_gw(m1, s1, 1)
            gw2 = make_gw(m2, s2, 2)

            # ---- dense phase: h = x @ W slab, write bf16 table slab ----
            def dense_layer(x_block_fn, w_bf, w_cols, slab_dram, kdim):
                """x_block_fn(b) -> SBUF f32 tile [128, kdim]; writes slab."""
                for b in range(BLOCKS_PER_CORE):
                    xb = x_block_fn(b)
                    xt_ps = pstpool.tile([kdim, 128], F32, tag="tps")
                    nc.tensor.transpose(xt_ps[:], xb, ident[:])
                    xt = wpool.tile([kdim, 128], BF16, tag="xt")
                    nc.scalar.copy(xt[:], xt_ps[:])
                    h_ps = pstpool.tile([128, w_cols], F32, tag="hps")
                    nc.tensor.matmul(h_ps[:], xt[:], w_bf[:], start=True, stop=True)
                    st = wpool.tile([128, TBL_COLS], BF16, tag="slabt")
                    nc.scalar.memzero(st[:])
                    nc.scalar.copy(st[:, 0:w_cols], h_ps[:])
                    rows = min(128, NODES_PER_CORE - b * 128)
                    nc.sync.dma_start(
                        slab_dram[b * 128 : b * 128 + rows, :], st[0:rows, :]
                    )

            # layer-1 dense: x = features
            def feat_block(b):
                ft = wpool.tile([128, IN_FEATS], F32, tag="feat")
                nc.sync.dma_start(ft[:], feat_ext[b * 128 : (b + 1) * 128, :])
                return ft[:]

            dense_layer(feat_block, w1b, N_HIDDEN, slab1, 128)

            def allgather(slab_dram, table_dram):
                if timing_mode:
                    # timing-only stand-in (TimelineSim is single-core):
                    # local copy approximating the AG wall time contribution
                    nc.sync.dma_start(
                        table_dram[0:NODES_PER_CORE, :], slab_dram[:]
                    )
                    return
                nc.gpsimd.collective_compute(
                    "AllGather",
                    mybir.AluOpType.bypass,
                    ins=[slab_dram[:]],
                    outs=[table_dram[:]],
                    replica_groups=[list(range(N_CORES))],
                )

            allgather(slab1, table1)

            # ---- edge phase ----
            def edge_layer(table_dram, gw, acc, w_cols, lname):
                # per pass: lo chunks [0, C_lo), hi chunks [C_lo, C)
                chunk0 = 0
                for p in range(2):
                    pass_chunks = C_lo if p == 0 else plan["C"] - C_lo
                    tbl = table_dram[p * HALF : p * HALF + HALF, :]
                    # gather batches
                    gtiles = []
                    bch = GBATCH // 128
                    nb = (pass_chunks + bch - 1) // bch
                    for k in range(nb):
                        cs = chunk0 + k * bch
                        nch = min(bch, chunk0 + pass_chunks - cs)
                        g = gpool.tile([128, bch, TBL_COLS], BF16, tag="g")
                        nidx = nch * 128
                        nc.gpsimd.dma_gather(
                            g[:, 0:nch, :],
                            tbl,
                            sidx[:, cs * 8 : cs * 8 + nidx // 16],
                            nidx,
                            nidx,
                            TBL_COLS,
                            queue_num=0,
                            single_packet=False,
                        )
                        gtiles.append(g)
                    # blocks
                    ci = chunk0
                    for b in range(BLOCKS_PER_CORE):
                        nch = int(gchunks[p * BLOCKS_PER_CORE + b])
                        ps = pspool.tile([128, w_cols], F32, tag="scat")
                        for j in range(nch):
                            c = ci + j
                            oh = ohpool.tile([128, 128], BF16, tag="oh")
                            nc.vector.tensor_scalar(
                                oh[:],
                                iota[:],
                                dstc[:, c : c + 1],
                                gw[:, c : c + 1],
                                mybir.AluOpType.is_equal,
                                mybir.AluOpType.mult,
                            )
                            krel = c - chunk0
                            g = gtiles[krel // (GBATCH // 128)]
                            nc.tensor.matmul(
                                ps[:],
                                oh[:],
                                g[:, krel % (GBATCH // 128), 0:w_cols],
                                start=(j == 0),
                                stop=(j == nch - 1),
                            )
                        if p == 0:
                            nc.scalar.copy(acc[:, b, :], ps[:])
                        else:
                            nc.vector.tensor_tensor(
                                acc[:, b, :], acc[:, b, :], ps[:], mybir.AluOpType.add
                            )
                        ci += nch
                    chunk0 += pass_chunks

            edge_layer(table1, gw1, acc1, N_HIDDEN, "l1")

            # ---- layer-2 dense ----
            def x2_block(b):
                xb = wpool.tile([128, N_HIDDEN], F32, tag="x2")
                nc.vector.tensor_tensor(xb[:], acc1[:, b, :], b1b[:], mybir.AluOpType.add)
                return xb[:]

            dense_layer(x2_block, w2b, OUT_FEATS, slab2, N_HIDDEN)

            allgather(slab2, table2)

            edge_layer(table2, gw2, acc2, OUT_FEATS, "l2")

            # ---- bias + log_softmax + output (batched per-phase so the
            # ACT table is not thrashed between Exp and Ln) ----
            NB = BLOCKS_PER_CORE
            o_all = apool.tile([128, NB, OUT_FEATS], F32)
            mxs = apool.tile([128, NB], F32)
            nmxs = apool.tile([128, NB], F32)
            ses = apool.tile([128, NB], F32)
            lses = apool.tile([128, NB], F32)
            for b in range(NB):
                nc.vector.tensor_tensor(
                    o_all[:, b, :], acc2[:, b, :], b2b[:], mybir.AluOpType.add
                )
                nc.vector.tensor_reduce(
                    mxs[:, b : b + 1], o_all[:, b, :],
                    mybir.AxisListType.X, mybir.AluOpType.max,
                )
            nc.vector.tensor_scalar(nmxs[:], mxs[:], -1.0, None, mybir.AluOpType.mult)
            for b in range(NB):
                ex = wpool.tile([128, OUT_FEATS], F32, tag="ex")
                nc.scalar.activation(
                    ex[:], o_all[:, b, :], mybir.ActivationFunctionType.Exp,
                    bias=nmxs[:, b : b + 1], scale=1.0, accum_out=ses[:, b : b + 1],
                )
            nc.scalar.activation(lses[:], ses[:], mybir.ActivationFunctionType.Ln)
            for b in range(NB):
                res = wpool.tile([128, OUT_FEATS], F32, tag="res")
                nc.vector.tensor_scalar(
                    res[:], o_all[:, b, :], mxs[:, b : b + 1], lses[:, b : b + 1],
                    mybir.AluOpType.subtract, mybir.AluOpType.subtract,
                )
                rows = min(128, NODES_PER_CORE - b * 128)
                nc.sync.dma_start(out_ext[b * 128 : b * 128 + rows, :], res[0:rows, :])

    nc.compile()
    return nc


def _make_in_maps(inp, plan, srcidx, dst_col, negdst, ew_wrapped):
    features = np.asarray(inp["features"], np.float32)
    W1 = np.asarray(inp["W1"], np.float32).reshape(IN_FEATS, N_HIDDEN)
    W2 = np.asarray(inp["W2"], np.float32).reshape(N_HIDDEN, OUT_FEATS)
    b1 = np.asarray(inp["b1"], np.float32).reshape(N_HIDDEN)
    b2 = np.asarray(inp["b2"], np.float32).reshape(OUT_FEATS)

    iota_np = np.tile(np.arange(128, dtype=np.float32), (128, 1)).astype(BF16NP)
    ident_np = np.eye(128, dtype=np.float32)
    b1b = np.tile(b1, (128, 1)).astype(np.float32)
    b2b = np.tile(b2, (128, 1)).astype(np.float32)
    m1 = np.asarray(inp["mu1"], np.float64).reshape(DIM)
    s1 = np.asarray(inp["inv_sigma1"], np.float64).reshape(DIM) / np.sqrt(2.0)
    m2 = np.asarray(inp["mu2"], np.float64).reshape(DIM)
    s2 = np.asarray(inp["inv_sigma2"], np.float64).reshape(DIM) / np.sqrt(2.0)
    gwb_row = np.concatenate([-m1 * s1, -m2 * s2]).astype(np.float32)
    gwb_np = np.tile(gwb_row, (128, 1)).astype(np.float32)

    in_maps = []
    for c in range(N_CORES):
        fs = np.zeros((NODES_PAD, IN_FEATS), np.float32)
        fs[:NODES_PER_CORE] = features[c * NODES_PER_CORE : (c + 1) * NODES_PER_CORE]
        in_maps.append(
            {
                "feat": fs,
                "ew": ew_wrapped[c],
                "dstc": dst_col[c],
                "sidx": srcidx[c],
                "w1": W1,
                "w2": W2,
                "b1b": b1b,
                "b2b": b2b,
                "iota": iota_np,
                "ident": ident_np,
                "gwb": gwb_np,
            }
        )
    return in_maps


def kernel(
    features,
    edge_weight,
    src,
    dst,
    W1,
    b1,
    mu1,
    inv_sigma1,
    W2,
    b2,
    mu2,
    inv_sigma2,
):
    inp = {
        "features": features,
        "W1": W1,
        "W2": W2,
        "b1": b1,
        "b2": b2,
        "mu1": mu1,
        "inv_sigma1": inv_sigma1,
        "mu2": mu2,
        "inv_sigma2": inv_sigma2,
    }
    plan, src_adj, srcidx, dst_col, negdst, ew_wrapped = _preprocess(src, dst, edge_weight)
    nc = _build_program_cached(plan, mu1, inv_sigma1, mu2, inv_sigma2)
    in_maps = _make_in_maps(inp, plan, srcidx, dst_col, negdst, ew_wrapped)
    res = run_bass_kernel_spmd(nc, in_maps, list(range(N_CORES)))
    out = np.concatenate([res.results[c]["out"] for c in range(N_CORES)], axis=0)
    return out.astype(np.float32)
